# revision 6
# baseline (speedup 1.0000x reference)
"""Trainium2 Bass kernel for the BDH dense-transformer problem.

Sharding: data-parallel over B=8 across the 8 NeuronCores (one batch
element per core, no collectives). Each core runs the full 6-layer
network on its [T=2048, D=256] slice.

Per-core program. Matmul precision strategy:
  - the per-layer FLOP bulk (attention energy/a, MLP x/y/update) runs
    in float32r: 1 cyc/row on the PE when the output free dim is
    >= 256 (true for all matmuls here), vs 4 cyc/row for fp32 and
    3 cyc/row for the bf16x2 3-pass split scheme. No host splits and
    no DVE split work. All tensors feeding an f32r matmul are declared
    float32r so writes round appropriately (BIR verifier requirement).
  - precision recovery: the residual stream vN and the update
    accumulator updW stay full fp32 (vNr is a rounded F32R mirror used
    only as the attention a-matmul input), and the run-once embedding
    and readout matmuls are full fp32. Only per-layer matmul-input
    roundings remain.
Structure:
  - token embedding via one-hot matmul (iota + is_equal + PE)
  - v kept in both layouts: vT [D,T] (f32r) and vN [T,D] (fp32)
  - causal linear attention block-wise: energyT = qr@qr^T per
    [s128, t512] block (PSUM), bf16-mask multiply, then aN accumulated
    in PSUM over s-chunks
  - LayerNorms in natural layout with fused ACT Square/Identity
    (per-partition scale+bias + accum_out row sums)
  - MLP streamed over N in eighths (fp32 weights DMA'd per layer in
    host-pre-shuffled partition-contiguous layouts),
    relu(x)*relu(y) fused via scalar_tensor_tensor, update accumulated
    in PSUM then SBUF
  - PE 128x128 transposes maintain both v layouts
"""

import math

import numpy as np
import ml_dtypes

import concourse.bass as bass
import concourse.tile as tile
from concourse import bacc, mybir
from concourse import bass_utils

F32 = mybir.dt.float32
F32R = mybir.dt.float32r
BF16 = mybir.dt.bfloat16
I32 = mybir.dt.int32
ALU = mybir.AluOpType
ACTF = mybir.ActivationFunctionType
AXX = mybir.AxisListType.X

B, T, D, N, H, VOCAB, L = 8, 2048, 256, 8192, 4, 256, 6
EPS = 1e-5
TS = 512          # t-super width
NSUP = T // TS    # 4
NTB = T // 128    # 16
NQ = 8            # weight chunks along N
NCHQ = N // 128 // NQ  # 8 n-chunks per weight chunk


def build_nc(layers=L, stream_weights=True, attn=True, cphase=True):
    nc = bacc.Bacc("TRN2", target_bir_lowering=False, debug=False)

    idx_d = nc.dram_tensor("idxf", [1, T], F32R, kind="ExternalInput")
    wte_d = nc.dram_tensor("wte", [VOCAB, D], F32, kind="ExternalInput")
    wx_d = nc.dram_tensor("wx", [128, 2, N], F32R, kind="ExternalInput")
    wy_d = nc.dram_tensor("wy", [128, 2, N], F32R, kind="ExternalInput")
    enc_d = nc.dram_tensor("enc", [128, N // 128, D], F32R, kind="ExternalInput")
    ro_d = nc.dram_tensor("ro", [D, VOCAB], F32, kind="ExternalInput")
    cos_d = nc.dram_tensor("cosT", [128, T], F32, kind="ExternalInput")
    sin_d = nc.dram_tensor("sinT", [128, T], F32, kind="ExternalInput")
    mask_d = nc.dram_tensor("maskbig", [128, 1024], BF16, kind="ExternalInput")
    ident_d = nc.dram_tensor("identm", [128, 128], F32, kind="ExternalInput")
    out_d = nc.dram_tensor("logits", [T, VOCAB], F32, kind="ExternalOutput")

    wx_r, wy_r, enc_r = wx_d.ap(), wy_d.ap(), enc_d.ap()
    wte_r = wte_d.ap().rearrange("(c p) d -> p c d", p=128)
    ro_r = ro_d.ap().rearrange("(c p) d -> p c d", p=128)

    with tile.TileContext(nc) as tc:
        with tc.tile_pool(name="persist", bufs=1) as pp, \
             tc.tile_pool(name="wq", bufs=2) as wq, \
             tc.tile_pool(name="blk", bufs=4) as blkp, \
             tc.tile_pool(name="sc", bufs=7) as scp, \
             tc.tile_pool(name="st", bufs=32) as stp, \
             tc.tile_pool(name="ps512", bufs=4, space="PSUM") as ps512, \
             tc.tile_pool(name="ps256", bufs=4, space="PSUM") as ps256:

            vT = [pp.tile([128, T], F32R, name=f"vT{c}", tag=f"vT{c}") for c in range(2)]
            vN = pp.tile([128, NTB, D], F32, name="vN", tag="vN")
            vNr = pp.tile([128, NTB, D], F32R, name="vNr", tag="vNr")
            qrT = [pp.tile([128, T], F32R, name=f"qrT{c}", tag=f"qrT{c}") for c in range(2)]
            lnaT = [pp.tile([128, T], F32R, name=f"lnaT{c}", tag=f"lnaT{c}") for c in range(2)]
            updF = pp.tile([128, NTB * D], F32, name="updF", tag="updF")
            _updv = updF.rearrange("p (o d) -> p o d", d=D)

            def updA(tb):
                return _updv[:, tb, :]
            cosT = pp.tile([128, T], F32, name="cosT", tag="cosT")
            sinT = pp.tile([128, T], F32, name="sinT", tag="sinT")
            maskb = pp.tile([128, 1024], BF16, name="maskb", tag="maskb")

            ident = pp.tile([128, 128], F32, name="ident", tag="ident")
            iota_f = pp.tile([128, 2], F32, name="iota_f", tag="iota_f")

            nc.sync.dma_start(cosT[:], cos_d.ap())
            nc.sync.dma_start(sinT[:], sin_d.ap())
            nc.sync.dma_start(maskb[:], mask_d.ap())

            nc.sync.dma_start(ident[:], ident_d.ap())

            copy_flip = [0]

            def copy_any(dst, src):
                # alternate PSUM->SBUF copies between ACT and DVE
                copy_flip[0] ^= 1
                if copy_flip[0]:
                    nc.scalar.copy(dst, src)
                else:
                    nc.vector.tensor_copy(dst, src)

            def mm(psum, lhsT, rhs, start, stop):
                nc.tensor.matmul(psum, lhsT, rhs, start=start, stop=stop)

            def tr128(dst, src):
                pst = ps512.tile([128, 512], F32, name="pst", tag="ps512")
                nc.tensor.transpose(pst[:, :128], src, ident[:])
                copy_any(dst, pst[:, :128])

            def ln_nat(src, dst, sums=None):
                """LayerNorm over free dim (256) of [128, 256] src -> dst.

                src may be PSUM or SBUF. sums = optional precomputed row sums.
                """
                if sums is None:
                    sums = stp.tile([128, 1], F32, name="s1", tag="st")
                    nc.vector.reduce_sum(sums, src, axis=AXX)
                negmean = stp.tile([128, 1], F32, name="negmean", tag="st")
                nc.vector.tensor_scalar_mul(negmean, sums, -1.0 / D)
                sq = scp.tile([128, D], F32, name="sq", tag="sc")
                sqs = stp.tile([128, 1], F32, name="sqs", tag="st")
                nc.scalar.activation(sq, src, ACTF.Square, bias=negmean, scale=1.0,
                                     accum_out=sqs)
                veps = stp.tile([128, 1], F32, name="veps", tag="st")
                nc.vector.tensor_scalar(veps, sqs, 1.0 / D, EPS, op0=ALU.mult, op1=ALU.add)
                sqv = stp.tile([128, 1], F32, name="sqv", tag="st")
                nc.scalar.sqrt(sqv, veps)
                rstd = stp.tile([128, 1], F32, name="rstd", tag="st")
                nc.vector.reciprocal(rstd, sqv)
                negmurs = stp.tile([128, 1], F32, name="negmurs", tag="st")
                nc.vector.tensor_tensor(negmurs, negmean, rstd, op=ALU.mult)
                nc.scalar.activation(dst, src, ACTF.Identity, bias=negmurs, scale=rstd)

            # ---------------- embedding: v = ln(wte[idx]) ----------------
            iota_i = pp.tile([128, 2], I32, name="iota_i", tag="iota_i")
            for c in range(2):
                nc.gpsimd.iota(iota_i[:, c:c + 1], pattern=[[1, 1]], base=c * 128,
                               channel_multiplier=1)
            nc.vector.tensor_copy(iota_f[:], iota_i[:])
            idx_b = lnaT[0]  # scratch alias
            nc.sync.dma_start(idx_b[:], idx_d.ap().partition_broadcast(128))
            onehot = [updF[:, 0:T], updF[:, T:2 * T]]  # scratch alias (F32)
            for c in range(2):
                nc.vector.tensor_scalar(onehot[c], idx_b[:], iota_f[:, c:c + 1], None,
                                        op0=ALU.is_equal)
            wte_s = blkp.tile([128, 2, D], F32, name="wte_s", tag="blk")
            nc.sync.dma_start(wte_s[:], wte_r)
            for tb in range(NTB):
                psA = ps256.tile([128, D], F32, name="psE", tag="ps256")
                for c in range(2):
                    mm(psA, onehot[c][:, tb * 128:(tb + 1) * 128], wte_s[:, c, :],
                       start=(c == 0), stop=(c == 1))
                ln_nat(psA, vN[:, tb, :])
                nc.vector.tensor_copy(vNr[:, tb, :], vN[:, tb, :])
                for c in range(2):
                    tr128(vT[c][:, tb * 128:(tb + 1) * 128], vN[:, tb, c * 128:(c + 1) * 128])

            # ---------------- layers ----------------
            if not stream_weights:
                wxq0 = wq.tile([128, 2, N // NQ], F32R, name="wxq", tag="wxq")
                nc.sync.dma_start(wxq0[:], wx_r[:, :, 0:N // NQ])
                wyq0 = wq.tile([128, 2, N // NQ], F32R, name="wyq", tag="wyq")
                nc.sync.dma_start(wyq0[:], wy_r[:, :, 0:N // NQ])
                encq0 = wq.tile([128, NCHQ, D], F32R, name="encq", tag="encq")
                nc.sync.dma_start(encq0[:], enc_r[:, 0:NCHQ, :])
            for layer in range(layers):
                # --- rope: qrT = vT*cos +/- rot*sin ---
                rsc = lnaT[1]  # dead scratch at this point
                nc.vector.tensor_tensor(qrT[0][:], vT[0][:], cosT[:], op=ALU.mult)
                nc.vector.tensor_tensor(rsc[:], vT[1][:], sinT[:], op=ALU.mult)
                nc.vector.tensor_tensor(qrT[0][:], qrT[0][:], rsc[:], op=ALU.subtract)
                nc.vector.tensor_tensor(qrT[1][:], vT[1][:], cosT[:], op=ALU.mult)
                nc.vector.tensor_tensor(rsc[:], vT[0][:], sinT[:], op=ALU.mult)
                nc.vector.tensor_tensor(qrT[1][:], qrT[1][:], rsc[:], op=ALU.add)

                # --- attention + LN(a) -> lnaT ---
                for si in range(NSUP if attn else 0):
                    psA = [ps256.tile([128, D], F32, name="psA", tag="ps256")
                           for _ in range(4)]
                    for sc in range(4 * si + 4):
                        psE = ps512.tile([128, TS], F32, name="psE", tag="ps512")
                        for c in range(2):
                            mm(psE, qrT[c][:, sc * 128:(sc + 1) * 128],
                               qrT[c][:, si * TS:(si + 1) * TS],
                               start=(c == 0), stop=(c == 1))
                        eT = blkp.tile([128, TS], F32R, name="eT", tag="blk")
                        k = sc - 4 * si
                        if k < 0:
                            copy_any(eT[:], psE[:])
                        else:
                            nc.vector.tensor_tensor(
                                eT[:], psE[:], maskb[:, 384 - k * 128: 896 - k * 128],
                                op=ALU.mult)
                        for tb4 in range(4):
                            tb = si * 4 + tb4
                            if sc <= tb:
                                mm(psA[tb4], eT[:, tb4 * 128:(tb4 + 1) * 128],
                                   vNr[:, sc, :], start=(sc == 0), stop=(sc == tb))
                    for tb4 in range(4):
                        tb = si * 4 + tb4
                        lna_n = scp.tile([128, D], F32, name="lna_n", tag="sc")
                        ln_nat(psA[tb4], lna_n)
                        for c in range(2):
                            tr128(lnaT[c][:, tb * 128:(tb + 1) * 128],
                                  lna_n[:, c * 128:(c + 1) * 128])

                # --- MLP over N eighths ---
                upd_sums = {}
                for q in range(NQ):
                    if stream_weights:
                        qs = slice(q * (N // NQ), (q + 1) * (N // NQ))
                        wxq = wq.tile([128, 2, N // NQ], F32R, name="wxq", tag="wxq")
                        nc.sync.dma_start(wxq[:], wx_r[:, :, qs])
                        wyq = wq.tile([128, 2, N // NQ], F32R, name="wyq", tag="wyq")
                        nc.sync.dma_start(wyq[:], wy_r[:, :, qs])
                        encq = wq.tile([128, NCHQ, D], F32R, name="encq", tag="encq")
                        nc.sync.dma_start(encq[:], enc_r[:, q * NCHQ:(q + 1) * NCHQ, :])
                    else:
                        wxq, wyq, encq = wxq0, wyq0, encq0
                    for si in range(NSUP):
                        sl = slice(si * TS, (si + 1) * TS)
                        ln_src = lnaT if attn else qrT
                        psU = [ps256.tile([128, D], F32, name="psU", tag="ps256")
                               for _ in range(4)]
                        for nch in range(NCHQ):
                            psX = ps512.tile([128, TS], F32, name="psX", tag="ps512")
                            psY = ps512.tile([128, TS], F32, name="psY", tag="ps512")
                            ns = slice(nch * 128, (nch + 1) * 128)
                            for c in range(2):
                                mm(psX, wxq[:, c, ns], vT[c][:, sl],
                                   start=(c == 0), stop=(c == 1))
                                mm(psY, wyq[:, c, ns], ln_src[c][:, sl],
                                   start=(c == 0), stop=(c == 1))
                            xr = blkp.tile([128, TS], F32, name="xr", tag="blk")
                            nc.scalar.activation(xr, psX, ACTF.Relu)
                            ysb = blkp.tile([128, TS], F32R, name="ysb", tag="blk")
                            nc.vector.scalar_tensor_tensor(
                                ysb, psY, 0.0, xr, op0=ALU.max, op1=ALU.mult)
                            for tb4 in range(4):
                                t4 = slice(tb4 * 128, (tb4 + 1) * 128)
                                mm(psU[tb4], ysb[:, t4], encq[:, nch, :],
                                   start=(nch == 0), stop=(nch == NCHQ - 1))
                        for tb4 in range(4):
                            tb = si * 4 + tb4
                            dst = updA(tb)
                            if q == 0:
                                nc.scalar.copy(dst, psU[tb4])
                            elif q < NQ - 1:
                                nc.vector.tensor_tensor(dst, psU[tb4], dst, op=ALU.add)
                            else:
                                s2 = stp.tile([128, 1], F32, name="s2", tag="st")
                                nc.vector.scalar_tensor_tensor(
                                    dst, psU[tb4], 0.0, dst, op0=ALU.add, op1=ALU.add,
                                    accum_out=s2)
                                upd_sums[tb] = s2

                # --- v = ln(v + ln(update)); maintain both layouts ---
                for tb in range(NTB if cphase else 0):
                    upd = updA(tb)
                    lnu = scp.tile([128, D], F32, name="lnu", tag="sc")
                    ln_nat(upd, lnu, sums=upd_sums[tb])
                    vmid = scp.tile([128, D], F32, name="vmid", tag="sc")
                    s3 = stp.tile([128, 1], F32, name="s3", tag="st")
                    nc.vector.scalar_tensor_tensor(
                        vmid, lnu, 0.0, vN[:, tb, :], op0=ALU.add, op1=ALU.add,
                        accum_out=s3)
                    ln_nat(vmid, vN[:, tb, :], sums=s3)
                    if layer < layers - 1:
                        nc.vector.tensor_copy(vNr[:, tb, :], vN[:, tb, :])
                        for c in range(2):
                            tr128(vT[c][:, tb * 128:(tb + 1) * 128],
                                  vN[:, tb, c * 128:(c + 1) * 128])

            # ---------------- readout (fp32: vv = v^T via PE transpose) ----
            ro_s = blkp.tile([128, 2, D], F32, name="ro_s", tag="blk")
            nc.sync.dma_start(ro_s[:], ro_r)
            for tb in range(NTB):
                vv = scp.tile([128, 2, 128], F32, name="vv", tag="sc")
                for c in range(2):
                    tr128(vv[:, c, :], vN[:, tb, c * 128:(c + 1) * 128])
                psR = ps256.tile([128, D], F32, name="psR", tag="ps256")
                for c in range(2):
                    mm(psR, vv[:, c, :], ro_s[:, c, :],
                       start=(c == 0), stop=(c == 1))
                lo = scp.tile([128, VOCAB], F32, name="lo", tag="sc")
                copy_any(lo[:], psR[:])
                nc.sync.dma_start(out_d.ap()[tb * 128:(tb + 1) * 128, :], lo[:])

    nc.compile()
    return nc


_NC_CACHE = {}


def get_nc():
    if "nc" not in _NC_CACHE:
        _NC_CACHE["nc"] = build_nc()
    return _NC_CACHE["nc"]


def make_host_inputs(idx, wte, encoder, decoder_x, decoder_y, readout):
    idx = np.asarray(idx)
    wte = np.asarray(wte, dtype=np.float32)
    encoder = np.asarray(encoder, dtype=np.float32)
    decoder_x = np.asarray(decoder_x, dtype=np.float32)
    decoder_y = np.asarray(decoder_y, dtype=np.float32)
    readout = np.asarray(readout, dtype=np.float32)

    wx = decoder_x.transpose(1, 0, 2).reshape(D, N)
    wy = decoder_y.transpose(1, 0, 2).reshape(D, N)
    # partition-contiguous layouts for fast DMA: [p, c, n] with d = c*128 + p
    wx = np.ascontiguousarray(wx.reshape(2, 128, N).transpose(1, 0, 2))
    wy = np.ascontiguousarray(wy.reshape(2, 128, N).transpose(1, 0, 2))
    # enc: [p, o, d] with n = o*128 + p
    enc_s = np.ascontiguousarray(encoder.reshape(N // 128, 128, D).transpose(1, 0, 2))

    inv_freq = 1.0 / (10000.0 ** (np.arange(0, D, 2, dtype=np.float32) / D))  # [128]
    t = np.arange(T, dtype=np.float32)
    freqsT = inv_freq[:, None] * t[None, :]                   # [128, T]
    cosT = np.cos(freqsT).astype(np.float32)
    sinT = np.sin(freqsT).astype(np.float32)

    s_idx = np.arange(128, dtype=np.int32)[:, None]
    c_idx = np.arange(1024, dtype=np.int32)[None, :]
    maskbig = (s_idx <= c_idx - 384).astype(ml_dtypes.bfloat16)

    in_maps = []
    for b in range(B):
        in_maps.append({
            "idxf": idx[b].astype(np.float32).reshape(1, T),
            "wte": wte,
            "wx": wx,
            "wy": wy,
            "enc": enc_s,
            "ro": readout,
            "cosT": cosT,
            "sinT": sinT,
            "maskbig": maskbig,
            "identm": np.eye(128, dtype=np.float32),
        })
    return in_maps


def kernel(idx, wte, encoder, decoder_x, decoder_y, readout):
    nc = get_nc()
    in_maps = make_host_inputs(idx, wte, encoder, decoder_x, decoder_y, readout)
    res = bass_utils.run_bass_kernel_spmd(nc, in_maps, core_ids=list(range(B)))
    out = np.stack([res.results[b]["logits"] for b in range(B)], axis=0)
    return out.astype(np.float32)
